# revision 1
# baseline (speedup 1.0000x reference)
"""Trainium2 Bass kernel for conditional-adjustment conv (CAConv).

Per sample b: h = relu(c[b] @ mlp_w1 + mlp_b1); adj = h @ mlp_w2 + mlp_b2;
w[b] = conv_w + adj.reshape(Co,Ci,3,3); out[b] = conv2d(x[b], w[b], pad=1) + conv_b.

Sharding: data-parallel over batch, 4 samples per core on 8 cores (SPMD).

Per-core device kernel (all big matmuls in float32r — full-rate fp32 on PE):
  Stage A (weight gen): one small matmul + relu produces hT[17,4]; the 17th
  row is ones (host-appended ones rows in c/w1), so row 16 of w2p — which
  the host sets to mlp_b2 + conv_w, both permuted — rides along and the
  scattered result is directly the complete per-sample conv weight.
  adj[b, t, ci, co] = hT.T @ w2p in 9 tap-chunks; each sample's [ci, co]
  block is DMA-scattered (SWDGE) into the diagonal blocks of the per-pair
  block-diagonal weight tile wblk[ci + 64*half, t*128 + 64*half + co],
  whose off-diagonal zeros come from a broadcast-DMA of a zeros vector.
  Stage B (conv): host-padded x (130x130) for a sample pair lives as
  [ci(2 samples), h, w] across the 128 partitions. Each output chunk
  po[128, 512] (2 samples x 64 co partitions; 4 h-rows x 128 w free)
  accumulates 9 shift-tap K=128 matmuls — one matmul per tap covers both
  samples. Bias is added during the PSUM->SBUF copy, then DMA to DRAM.

  DMA queues: sync HWDGE = consts + w2 chunks + output stores (small or
  late - keeps stage-A loads low-latency); ACT HWDGE = the two bulk x
  loads (128-partition, descriptor-size-capped so queue round-robin stays
  fair); GPSIMD SWDGE = weight scatters + zero-fills (latency-tolerant).
"""

import sys

if "/opt/trn_rl_repo" not in sys.path:
    sys.path.insert(0, "/opt/trn_rl_repo")

import numpy as np

B = 32
NCORES = 8
BPC = B // NCORES          # samples per core = 4
PAIRS = BPC // 2           # sample pairs per core = 2
CIN = COUT = 64
H = W = 128
HP = WP = 130              # padded dims
KH = KW = 3
NT = KH * KW               # taps = 9
CL = 8                     # c length
CL1 = CL + 1               # + ones row
MH = 16                    # mlp hidden
K2 = MH + 1                # mlp hidden + ones row
NCH = (H * W) // 512       # 512-col output chunks per pair = 32

_CACHE = {}


def _build():
    import concourse.bass as bass
    import concourse.mybir as mybir
    import concourse.tile as tile
    from concourse import bacc
    from concourse.tile_rust import add_dep_helper

    f32 = mybir.dt.float32
    f32r = mybir.dt.float32r
    AF = mybir.ActivationFunctionType

    nc = bacc.Bacc("TRN2", target_bir_lowering=False, debug=False)

    xs_d = nc.dram_tensor("xsp", [BPC, CIN, HP * WP], f32r, kind="ExternalInput")
    ct_d = nc.dram_tensor("cT", [CL1, BPC], f32, kind="ExternalInput")
    w1_d = nc.dram_tensor("w1", [CL1, K2], f32, kind="ExternalInput")
    b1_d = nc.dram_tensor("b1", [K2, 1], f32, kind="ExternalInput")
    w2_d = nc.dram_tensor("w2p", [128, (NT * CIN * COUT) // 4], f32r, kind="ExternalInput")
    zz_d = nc.dram_tensor("zz", [1, NT * 128], f32r, kind="ExternalInput")
    cb_d = nc.dram_tensor("cb2", [128, 1], f32, kind="ExternalInput")
    out_d = nc.dram_tensor("out", [BPC, COUT, H, W], f32, kind="ExternalOutput")

    with tile.TileContext(nc) as tc:
        with (
            tc.tile_pool(name="consts", bufs=1) as consts,
            tc.tile_pool(name="adjpool", bufs=2) as adjpool,
            tc.tile_pool(name="xpool", bufs=2) as xpool,
            tc.tile_pool(name="opool", bufs=6) as opool,
            tc.tile_pool(name="pspool", bufs=1, space=bass.MemorySpace.PSUM) as ps,
        ):
            # ---- constants in (sync queue; kept small + early) ----
            ct_sb = consts.tile([CL1, BPC], f32)
            nc.sync.dma_start(out=ct_sb[:], in_=ct_d.ap())
            w1_sb = consts.tile([CL1, K2], f32)
            nc.sync.dma_start(out=w1_sb[:], in_=w1_d.ap())
            b1_sb = consts.tile([K2, 1], f32)
            nc.sync.dma_start(out=b1_sb[:], in_=b1_d.ap())
            cb_sb = consts.tile([128, 1], f32)
            nc.sync.dma_start(out=cb_sb[:], in_=cb_d.ap())

            # ---- bulk x loads: one 128-partition DMA per pair (ACT queue),
            # descriptor size capped so other queues stay responsive ----
            # w2s shares the xpool slots: it occupies slot 0 during stage
            # A; xp1 (loaded during pair-0 conv) then reuses that slot.
            w2s = xpool.tile([128, (NT * CIN * COUT) // 4], f32r, name="w2s", tag="xp")
            for cc in range(4):
                nc.sync.dma_start(
                    out=w2s[:, cc * 2304 : (cc + 1) * 2304],
                    in_=w2_d.ap()[:, cc * 2304 : (cc + 1) * 2304],
                )
            xps = []
            for p in range(PAIRS):
                xp = xpool.tile([128, HP * WP], f32r, name=f"xp{p}", tag="xp")
                xps.append(xp)

            def load_x_chunk(p, k, after=None):
                # chunk = 13 padded rows; conv range-deps start on early
                # rows while the tail still streams. `after` paces the
                # bulk chunk behind latency-critical ring traffic.
                inst = nc.sync.dma_start(
                    out=xps[p][:, k * 1690 : (k + 1) * 1690],
                    in_=xs_d.ap()[2 * p : 2 * p + 2].rearrange(
                        "b c (k e) -> b c k e", e=1690
                    )[:, :, k, :],
                )
                if after is not None:
                    add_dep_helper(
                        after.ins, inst.ins, sync=True, reason="pace bulk x"
                    )
                return inst

            # per-pair block-diag weights; off-diag zero-filled via
            # broadcast DMA (SWDGE)
            wblk = []
            for p in range(PAIRS):
                wb = consts.tile([128, NT * 128], f32r, name=f"wblk{p}", tag=f"wblk{p}")
                zsrc = bass.AP(
                    tensor=zz_d.ap().tensor, offset=0, ap=[[0, 128], [1, NT * 128]]
                )
                nc.gpsimd.dma_start(out=wb[:], in_=zsrc)
                wblk.append(wb)

            # ---- stage A: conditioning MLP ----
            ph = ps.tile([K2, BPC], f32, tag="psA", bufs=2)
            nc.tensor.matmul(ph[:], w1_sb[:], ct_sb[:], start=True, stop=True)
            # hT replicated at partition offsets 0/32/64/96 to match the
            # packed w2 k-groups (matmul needs equal base partitions)
            ht_sb = consts.tile([128, BPC], f32r)
            nc.scalar.activation(
                out=ht_sb[0:K2, :], in_=ph[:], func=AF.Relu, bias=b1_sb[:]
            )
            for g in range(1, 4):
                nc.sync.dma_start(
                    out=ht_sb[32 * g : 32 * g + K2, :], in_=ht_sb[0:K2, :]
                )

            # adj[b, t, ci, co] = sum_k hT[k, b] w2p[k, t, ci, co]
            # (w2p row 16 carries mlp_b2 + conv_w, so adj is the full weight)
            scat_last = {}
            for t in range(NT):
                adj = adjpool.tile([BPC, CIN * COUT], f32r)
                for m in range(4):
                    pa = ps.tile([BPC, 1024], f32, tag="psA", bufs=2)
                    for n in range(2):
                        j = t * CIN * COUT + m * 1024 + n * 512
                        g, col = divmod(j, (NT * CIN * COUT) // 4)
                        nc.tensor.matmul(
                            pa[:, n * 512 : (n + 1) * 512],
                            ht_sb[32 * g : 32 * g + K2, :],
                            w2s[32 * g : 32 * g + K2, col : col + 512],
                            start=True,
                            stop=True,
                            tile_position=(32 * g, 0),
                        )
                    nc.any.tensor_copy(
                        adj[:, m * 1024 : (m + 1) * 1024],
                        pa[:],
                    )
                # scatter each sample's [ci, co] block onto wblk's diagonal
                # (split across both HWDGE engines)
                for b in range(BPC):
                    p, half = divmod(b, 2)
                    q = half * 64
                    scat_last[t] = nc.sync.dma_start(
                        out=wblk[p][q : q + 64, t * 128 + q : t * 128 + q + 64],
                        in_=adj[b : b + 1, :],
                    )

            last_out = scat_last[NT - 1]
            # ---- stage B: per-pair conv, tap-outer in groups of 4
            # chunks. A group's tap-t matmuls depend only on scatter(t) +
            # the x rows it reads, so conv work interleaves into stage-A
            # stalls instead of waiting for the last tap. ----
            for k in range(10):
                load_x_chunk(0, k, after=scat_last[min(k, NT - 1)])
            for p in range(PAIRS):
                xp3 = xps[p].rearrange("p (h w) -> p h w", w=WP)
                for g in range(NCH // 4):
                    if p + 1 < PAIRS and g < 5:
                        load_x_chunk(p + 1, 2 * g, after=last_out)
                        load_x_chunk(p + 1, 2 * g + 1, after=last_out)
                    pos = [
                        ps.tile([128, 512], f32, tag="ps", bufs=4, name=f"po{p}_{g}_{j}")
                        for j in range(4)
                    ]
                    for t in range(NT):
                        kh, kw = divmod(t, 3)
                        for j in range(4):
                            h0 = (g * 4 + j) * 4
                            nc.tensor.matmul(
                                pos[j][:],
                                wblk[p][:, t * 128 : (t + 1) * 128],
                                xp3[:, h0 + kh : h0 + kh + 4, kw : kw + W],
                                start=(t == 0),
                                stop=(t == NT - 1),
                            )
                    for j in range(4):
                        h0 = (g * 4 + j) * 4
                        os = opool.tile([128, 512], f32, name=f"os{p}_{g}_{j}", tag="os")
                        nc.vector.tensor_scalar_add(os[:], pos[j][:], cb_sb[:])
                        last_out = nc.sync.dma_start(
                            out=out_d.ap()[2 * p : 2 * p + 2, :, h0 : h0 + 4, :],
                            in_=os[:],
                        )

    nc.compile()
    return nc


def _get_nc():
    if "nc" not in _CACHE:
        _CACHE["nc"] = _build()
    return _CACHE["nc"]


def _prep(x, c, conv_w, conv_b, mlp_w1, mlp_b1, mlp_w2, mlp_b2):
    x = np.ascontiguousarray(x, dtype=np.float32)
    c = np.ascontiguousarray(c, dtype=np.float32)
    conv_w = np.asarray(conv_w, dtype=np.float32)
    conv_b = np.asarray(conv_b, dtype=np.float32)
    mlp_w1 = np.asarray(mlp_w1, dtype=np.float32)
    mlp_b1 = np.asarray(mlp_b1, dtype=np.float32)
    mlp_w2 = np.asarray(mlp_w2, dtype=np.float32)
    mlp_b2 = np.asarray(mlp_b2, dtype=np.float32)

    # padded x, flattened spatial
    xsp = np.zeros((B, CIN, HP, WP), dtype=np.float32)
    xsp[:, :, 1 : HP - 1, 1 : WP - 1] = x
    xsp = xsp.reshape(B, CIN, HP * WP)

    # w1' [CL1, K2]: [[w1, 0], [0, 1]]; cT' [CL1, BPC] gets a ones row
    w19 = np.zeros((CL1, K2), dtype=np.float32)
    w19[:CL, :MH] = mlp_w1
    w19[CL, MH] = 1.0
    b117 = np.concatenate([mlp_b1, np.zeros(1, np.float32)]).reshape(K2, 1)
    b117 = np.ascontiguousarray(b117, dtype=np.float32)

    # w2p[k, t, ci, co] = mlp_w2[k, co*576 + ci*9 + t]
    # row 16 = (mlp_b2 + conv_w), same permutation -> adj == full weight
    w2p = mlp_w2.reshape(MH, COUT, CIN, NT).transpose(0, 3, 2, 1)
    b2p = mlp_b2.reshape(COUT, CIN, NT).transpose(2, 1, 0)
    cwp = conv_w.reshape(COUT, CIN, NT).transpose(2, 1, 0)  # [t, ci, co]
    row16 = (b2p + cwp).reshape(1, -1)
    w2p = np.concatenate([w2p.reshape(MH, -1), row16], axis=0).astype(np.float32)
    w2g = w2p.reshape(K2, 4, (NT * CIN * COUT) // 4).transpose(1, 0, 2)
    w2pk = np.zeros((128, (NT * CIN * COUT) // 4), dtype=np.float32)
    for g in range(4):
        w2pk[32 * g : 32 * g + K2] = w2g[g]
    w2p = w2pk

    zz = np.zeros((1, NT * 128), dtype=np.float32)
    cb2 = np.ascontiguousarray(
        np.tile(conv_b.reshape(COUT, 1), (2, 1)), dtype=np.float32
    )

    in_maps = []
    for i in range(NCORES):
        sl = slice(i * BPC, (i + 1) * BPC)
        ct9 = np.concatenate([c[sl].T, np.ones((1, BPC), np.float32)], axis=0)
        in_maps.append(
            {
                "xsp": np.ascontiguousarray(xsp[sl]),
                "cT": np.ascontiguousarray(ct9),
                "w1": w19,
                "b1": b117,
                "w2p": w2p,
                "zz": zz,
                "cb2": cb2,
            }
        )
    return in_maps


def _run(inputs, trace=False):
    from concourse.bass_utils import run_bass_kernel_spmd

    nc = _get_nc()
    in_maps = _prep(**inputs)
    res = run_bass_kernel_spmd(
        nc, in_maps, core_ids=list(range(NCORES)), trace=trace
    )
    out = np.concatenate([res.results[i]["out"] for i in range(NCORES)], axis=0)
    return out, res


def kernel(**inputs):
    out, _ = _run(inputs, trace=False)
    return out

